# revision 7
# baseline (speedup 1.0000x reference)
"""InteractionMapInit Trainium2 kernel (v2).

out[i, j, :] = tanh( (X@Wt + bt)[i] - (Dft@Wd + bd)[j] + dnorm[i, j] )  if seg_res[i] == seg_atom[j]
             = 0                                                        otherwise

Block-diagonal over B=8 DT-pairs -> one block per NeuronCore (SPMD), host
scatters blocks into the zeros output.

v2 design (driven by dispatch-cost + roofline analysis):
  - ONE packed fp16 input tensor per core (was 11 tensors) and one fp16
    output tensor: per-exec dispatch cost scales with arg count and bytes.
  - All heavy matmuls in fp16 (full PE rate); distances in fp32 via a
    hi/lo fp16 split of the position tensors (fp16 pairs reconstructed
    on device to fp32 -- keeps D**2 cancellation error ~1e-4).
  - i4 (tiled identity) and r2d (block-broadcast matrix) generated on
    device with gpsimd affine_select instead of DMA'd.
  - Distances computed pre-transposed: D2T[j, i] = rhd.T @ lhd in one
    fp32 matmul; dnorm^T lands directly in the mm2 lhsT tile (no PE
    transposes of dnorm tiles).
  - Output rows trimmed to Ro = roundup(max block rows, 8) <= Rp.
  - tanh on ACT with fp16 output; sqrt/tanh table sets prefetched with
    dummy activations so the ~2.7us loads hide under the preamble.

Per-core device program (Rp padded rows for compute, Ro output rows,
Ap padded atoms):
  tfT  [H, Ro]   = Wt.T @ X.T + (bt - bd)      (fp16 matmuls, fp32 psum)
  df   [Ap, H]   = Dft.T @ Wd                  -> -df flat to r2d row Ap
  D2T  [Ap, Rp]  = rhd.T @ lhd  (fp32, 5-term |t-d|^2 trick), clamp >= 0, sqrt
  dmin/dmax via DVE reduces + one PE transpose; dnorm^T = (DT - dmin)/denom
  lhsT2 [Ap+1, Rp] = [dnorm^T; ones] fp16
  psum[i, (j,h)] = mm1(tfT, i4) + mm2(lhsT2, r2d); tanh -> fp16 -> DMA out
"""

import numpy as np

NR, NA, TD, DD, H, B = 3200, 320, 512, 128, 128, 8
NCORES = 8
P = 128

_last_results = None
_last_nc = None
_last_in_maps = None


def _pack_layout(Rp, Ro, Ap):
    """Two pack regions: A = [128, CA] (col ranges per piece), B = [5, CB]."""
    colsA = [("wd", H), ("dft", Ap), ("biasT", 1), ("wt", 4 * H), ("xt", 4 * Ro)]
    colsB = [("lhd_hi", Rp), ("lhd_lo", Rp), ("rhd_hi", Ap), ("rhd_lo", Ap)]
    layA, c = {}, 0
    for name, n in colsA:
        layA[name] = (c, n)
        c += n
    CA = c
    layB, c = {}, 0
    for name, n in colsB:
        layB[name] = (c, n)
        c += n
    CB = c
    total = P * CA + 5 * CB
    return layA, CA, layB, CB, total


def _host_prep(target_feature, drug_feature, target_pos, drug_pos,
               Wt, bt, Wd, bd, seg_res, seg_atom):
    f32, f16 = np.float32, np.float16
    X = np.asarray(target_feature, f32)
    Dft = np.asarray(drug_feature, f32)
    tp = np.asarray(target_pos, f32)
    dp = np.asarray(drug_pos, f32)
    Wt = np.asarray(Wt, f32)
    Wd = np.asarray(Wd, f32)
    bias = (np.asarray(bt, f32) - np.asarray(bd, f32)).reshape(H)
    seg_res = np.asarray(seg_res)
    seg_atom = np.asarray(seg_atom)

    r0 = np.searchsorted(seg_res, np.arange(B), side="left")
    r1 = np.searchsorted(seg_res, np.arange(B), side="right")
    a0 = np.searchsorted(seg_atom, np.arange(B), side="left")
    a1 = np.searchsorted(seg_atom, np.arange(B), side="right")
    r_cnt = (r1 - r0).astype(int)
    a_cnt = (a1 - a0).astype(int)

    Rp = max(P, int(-(-max(r_cnt) // P)) * P)      # compute row padding
    Ro = max(8, int(-(-max(r_cnt) // 8)) * 8)      # output row padding
    Ap = max(4, int(-(-max(a_cnt) // 4)) * 4)
    assert Ap + 1 <= 128

    layA, CA, layB, CB, total = _pack_layout(Rp, Ro, Ap)
    # wt region A layout: [p, (k, h)] with source row t = k*128 + p
    wt16 = np.ascontiguousarray(
        Wt.astype(f16).reshape(4, P, H).transpose(1, 0, 2).reshape(P, 4 * H))
    wd16 = Wd.astype(f16)
    bias16 = bias.astype(f16)

    def hi_lo(a):
        hi = a.astype(f16)
        lo = (a - hi.astype(f32)).astype(f16)
        return hi, lo

    in_maps = []
    for c in range(B):
        rc, ac = r_cnt[c], a_cnt[c]
        xt = np.zeros((TD, Ro), f32)
        dft = np.zeros((DD, Ap), f32)
        tpp = np.zeros((Rp, 3), f32)
        dpp = np.zeros((Ap, 3), f32)
        xt[:, :rc] = X[r0[c]:r1[c]].T
        tpp[:rc] = tp[r0[c]:r1[c]]
        tpp[rc:] = tp[r1[c] - 1]
        dft[:, :ac] = Dft[a0[c]:a1[c]].T
        dpp[:ac] = dp[a0[c]:a1[c]]
        dpp[ac:] = dp[a1[c] - 1]

        lhd = np.empty((5, Rp), f32)
        lhd[0:3] = tpp.T
        lhd[3] = 1.0
        lhd[4] = (tpp * tpp).sum(axis=1)
        rhd = np.empty((5, Ap), f32)
        rhd[0:3] = -2.0 * dpp.T
        rhd[3] = (dpp * dpp).sum(axis=1)
        rhd[4] = 1.0
        lhd_hi, lhd_lo = hi_lo(lhd)
        rhd_hi, rhd_lo = hi_lo(rhd)

        pack = np.empty(total, f16)
        A = pack[:P * CA].reshape(P, CA)
        Bv = pack[P * CA:].reshape(5, CB)

        def putA(name, arr):
            off, n = layA[name]
            A[:, off:off + n] = np.asarray(arr, f16).reshape(P, n)

        def putB(name, arr):
            off, n = layB[name]
            Bv[:, off:off + n] = np.asarray(arr, f16).reshape(5, n)

        putA("wd", wd16)
        putA("dft", dft)
        putA("biasT", bias16.reshape(P, 1))
        putA("wt", wt16)
        putA("xt", np.ascontiguousarray(
            xt.reshape(4, P, Ro).transpose(1, 0, 2).reshape(P, 4 * Ro)))
        putB("lhd_hi", lhd_hi)
        putB("lhd_lo", lhd_lo)
        putB("rhd_hi", rhd_hi)
        putB("rhd_lo", rhd_lo)
        in_maps.append({"pack": pack})

    meta = dict(r0=r0, a0=a0, r_cnt=r_cnt, a_cnt=a_cnt, Rp=Rp, Ro=Ro, Ap=Ap)
    return in_maps, meta


def build_bass(Rp, Ro, Ap):
    from contextlib import ExitStack

    import concourse.bacc as bacc
    import concourse.mybir as mybir
    import concourse.tile as tile
    from concourse.masks import make_identity

    F32 = mybir.dt.float32
    F16 = mybir.dt.float16
    AX = mybir.AxisListType
    OP = mybir.AluOpType
    AF = mybir.ActivationFunctionType

    K_TD = TD // P        # 4 contraction chunks for the target linear
    RT = Rp // P          # 128-row tiles
    AH = Ap * H
    NCH = AH // 512       # 512-wide psum chunks (4 atoms x H)
    GRP = 4               # chunks per psum group (4 banks; x2 groups = 8)

    layA, CA, layB, CB, total = _pack_layout(Rp, Ro, Ap)

    nc = bacc.Bacc("TRN2", target_bir_lowering=False, debug=False,
                   num_devices=NCORES)

    pack_d = nc.dram_tensor("pack", [total], F16, kind="ExternalInput").ap()
    out_d = nc.dram_tensor("out", [Ro, AH], F16, kind="ExternalOutput").ap()

    with tile.TileContext(nc) as tc, ExitStack() as ctx:
        singles = ctx.enter_context(tc.tile_pool(name="singles", bufs=1))
        temps = ctx.enter_context(tc.tile_pool(name="temps", bufs=2))
        psum = ctx.enter_context(tc.tile_pool(name="psum", bufs=2, space="PSUM"))
        outs = ctx.enter_context(tc.tile_pool(name="outs", bufs=3))
        dram = ctx.enter_context(tc.tile_pool(name="dram", bufs=1, space="DRAM"))

        # ---------------- inputs to SBUF: 2 consolidated DMAs ----------------
        packA = singles.tile([P, CA], F16, name="packA")
        nc.sync.dma_start(out=packA,
                          in_=pack_d[:P * CA].rearrange("(p c) -> p c", p=P))
        packB = singles.tile([5, CB], F16, name="packB")
        nc.gpsimd.dma_start(out=packB,
                            in_=pack_d[P * CA:].rearrange("(p c) -> p c", p=5))

        def pA(name):
            off, n = layA[name]
            return packA[:, off:off + n]

        def pB(name):
            off, n = layB[name]
            return packB[:, off:off + n]

        wd_sb = pA("wd")
        dft_sb = pA("dft")
        biasT = pA("biasT")
        wt_sb = pA("wt").rearrange("p (k h) -> p k h", h=H)
        xt_sb = pA("xt").rearrange("p (k i) -> p k i", i=Ro)
        lhd_hi = pB("lhd_hi")
        lhd_lo = pB("lhd_lo")
        rhd_hi = pB("rhd_hi")
        rhd_lo = pB("rhd_lo")

        # ---------------- on-device constants (cheap, off critical path) ----
        idn32 = singles.tile([P, P], F32, name="idn32")
        make_identity(nc, idn32)
        idn16 = singles.tile([P, P], F16, name="idn16")
        nc.gpsimd.memset(idn16, 0.0)
        nc.gpsimd.affine_select(
            out=idn16, in_=idn16, compare_op=OP.not_equal, fill=1.0, base=0,
            pattern=[[-1, P]], channel_multiplier=1)
        # mm1 rhs: rhs[h', (j, h)] = idn16[h', h]  (j is a stride-0 dim)
        i4_bc = idn16.rearrange("p (one h) -> p one h", one=1).broadcast_to([P, 4, P])

        # I48ext [Ap+1, Ap]: identity in rows 0..Ap-1, zero row Ap
        i48 = singles.tile([Ap + 1, Ap], F16, name="i48")
        nc.gpsimd.memset(i48, 0.0)
        nc.gpsimd.affine_select(
            out=i48[:Ap], in_=i48[:Ap], compare_op=OP.not_equal, fill=1.0, base=0,
            pattern=[[-1, Ap]], channel_multiplier=1)

        # r2d rows 0..Ap-1: r2d[j, (j',h)] = (j == j') via DVE broadcast copy;
        # row Ap: -df flat (DMA roundtrip below)
        r2d = singles.tile([Ap + 1, AH], F16, name="r2d")
        nc.vector.tensor_copy(
            out=r2d[:Ap].rearrange("j (jp h) -> j jp h", h=H),
            in_=i48[:Ap].rearrange("j (jp one) -> j jp one", one=1).broadcast_to([Ap, Ap, H]))

        # ---------------- ACT table prefetch ----------------
        ones_sb = singles.tile([1, 512], F32, name="ones_sb")
        nc.vector.memset(ones_sb, 1.0)
        scr = temps.tile([1, 16], F32, name="scr")
        nc.scalar.activation(out=scr, in_=ones_sb[:, :16], func=AF.Sqrt)

        # lhsT2: rows 0..Ap-1 get dnorm^T below; row Ap stays all-ones
        lhsT2 = singles.tile([Ap + 1, Rp], F16, name="lhsT2")
        nc.vector.memset(lhsT2, 1.0)

        # ---------------- df = Dft.T @ Wd ; -df -> r2d row Ap ----------------
        ps_df = psum.tile([P, GRP * 512], F32, tag="ps", name="ps_df")
        nc.tensor.matmul(ps_df[:Ap, :H], lhsT=dft_sb, rhs=wd_sb,
                         start=True, stop=True)
        dfneg = temps.tile([Ap, H], F16, name="dfneg")
        nc.vector.tensor_scalar_mul(dfneg, ps_df[:Ap, :H], -1.0)
        dscr = dram.tile([AH], F16, name="dscr")
        nc.gpsimd.dma_start(out=dscr.rearrange("(a h) -> a h", h=H), in_=dfneg)
        nc.gpsimd.dma_start(out=r2d[Ap:Ap + 1, :], in_=dscr[None, :])

        # ---------------- tfT = Wt.T @ X.T + bias  [H, Ro] ----------------
        biasT32 = temps.tile([P, 1], F32, name="biasT32")
        nc.vector.tensor_copy(out=biasT32, in_=biasT)
        ps_tf = psum.tile([P, GRP * 512], F32, tag="ps", name="ps_tf")
        for k in range(K_TD):
            nc.tensor.matmul(ps_tf[:, :Ro], lhsT=wt_sb[:, k, :],
                             rhs=xt_sb[:, k, :], start=(k == 0), stop=(k == K_TD - 1))
        tfT = singles.tile([P, Rp], F16, name="tfT")
        if Ro < Rp:
            nc.vector.memset(tfT[:, Ro:], 0.0)
        nc.vector.tensor_scalar(out=tfT[:, :Ro], in0=ps_tf[:, :Ro], scalar1=biasT32,
                                scalar2=None, op0=OP.add)

        # ---------------- distances: D2T = rhd.T @ lhd  [Ap, Rp] ----------------
        lhd_sb = temps.tile([5, Rp], F32, name="lhd_sb")
        nc.vector.tensor_tensor(out=lhd_sb, in0=lhd_hi, in1=lhd_lo, op=OP.add)
        rhd_sb = temps.tile([5, Ap], F32, name="rhd_sb")
        nc.vector.tensor_tensor(out=rhd_sb, in0=rhd_hi, in1=rhd_lo, op=OP.add)

        ps_d = psum.tile([P, GRP * 512], F32, tag="ps", name="ps_d")
        nc.tensor.matmul(ps_d[:Ap, :Rp], lhsT=rhd_sb, rhs=lhd_sb,
                         start=True, stop=True)
        dt2 = temps.tile([Ap, Rp], F32, name="dt2")
        nc.vector.tensor_scalar_max(dt2, ps_d[:Ap, :Rp], 0.0)
        dt = singles.tile([Ap, Rp], F32, name="dt")
        nc.scalar.activation(out=dt, in_=dt2, func=AF.Sqrt)
        # prefetch tanh table now (only remaining ACT set)
        scr2 = temps.tile([1, 16], F32, name="scr2")
        nc.scalar.activation(out=scr2, in_=dt[:1, :16], func=AF.Tanh)

        # per-block dmin/dmax
        stats = temps.tile([Ap, 2], F32, name="stats")
        nc.vector.tensor_reduce(out=stats[:, 0:1], in_=dt, axis=AX.X, op=OP.min)
        nc.vector.tensor_reduce(out=stats[:, 1:2], in_=dt, axis=AX.X, op=OP.max,
                                negate=True)
        ps_t1 = psum.tile([P, GRP * 512], F32, tag="ps", name="ps_t1")
        nc.tensor.transpose(ps_t1[:2, :Ap], stats, idn32[:Ap, :Ap])
        mm2c = temps.tile([2, 1], F32, name="mm2c")
        nc.vector.tensor_reduce(out=mm2c, in_=ps_t1[:2, :Ap], axis=AX.X, op=OP.min)
        ps_t2 = psum.tile([P, GRP * 512], F32, tag="ps", name="ps_t2")
        nc.tensor.transpose(ps_t2[:1, :2], mm2c, idn32[:2, :2])
        sc = temps.tile([1, 2], F32, name="sc")      # [dmin, -dmax]
        nc.vector.tensor_copy(out=sc, in_=ps_t2[:1, :2])

        diff = temps.tile([1, 1], F32, name="diff")   # dmax - dmin
        nc.vector.tensor_scalar(out=diff, in0=sc[:, 0:1], scalar1=sc[:, 1:2],
                                scalar2=-1.0, op0=OP.add, op1=OP.mult)
        denom = temps.tile([1, 1], F32, name="denom")
        nc.vector.tensor_scalar_max(denom, diff, 1e-30)
        inv = temps.tile([1, 1], F32, name="inv")
        nc.vector.reciprocal(out=inv, in_=denom)
        bv = temps.tile([1, 2], F32, name="bv")       # [dmin, 1/denom]
        nc.vector.tensor_copy(out=bv[:, 0:1], in_=sc[:, 0:1])
        nc.vector.tensor_copy(out=bv[:, 1:2], in_=inv)
        ps_b = psum.tile([P, GRP * 512], F32, tag="ps", name="ps_b")
        nc.tensor.matmul(ps_b[:Ap, :2], lhsT=ones_sb[:, :Ap], rhs=bv,
                         start=True, stop=True)
        cols = temps.tile([Ap, 2], F32, name="cols")
        nc.vector.tensor_copy(out=cols, in_=ps_b[:Ap, :2])

        # dnorm^T straight into lhsT2 rows 0..Ap-1
        nc.vector.tensor_scalar(out=lhsT2[:Ap, :], in0=dt,
                                scalar1=cols[:, 0:1], scalar2=cols[:, 1:2],
                                op0=OP.subtract, op1=OP.mult)

        # ---------------- main: psum = tf - df + dnorm ; tanh ; out ----------------
        for rt in range(RT):
            i_lo = rt * P
            m = min(Ro, i_lo + P) - i_lo          # output rows this tile
            if m <= 0:
                break
            l2_sl = lhsT2[:, i_lo:i_lo + P]
            tf_sl = tfT[:, i_lo:i_lo + P]
            for g in range(NCH // GRP):
                pso = psum.tile([P, GRP * 512], F32, tag="ps", name="pso")
                for c in range(GRP):
                    ch = g * GRP + c
                    csl = slice(512 * c, 512 * (c + 1))
                    nc.tensor.matmul(pso[:, csl], lhsT=tf_sl, rhs=i4_bc,
                                     start=True, stop=False)
                    nc.tensor.matmul(pso[:, csl], lhsT=l2_sl,
                                     rhs=r2d[:, 512 * ch:512 * (ch + 1)],
                                     start=False, stop=True)
                ob = outs.tile([P, GRP * 512], F16, name="ob")
                nc.scalar.activation(out=ob, in_=pso, func=AF.Tanh)
                eng = nc.sync if (rt * (NCH // GRP) + g) % 2 == 0 else nc.gpsimd
                eng.dma_start(
                    out=out_d[i_lo:i_lo + m, 512 * GRP * g:512 * GRP * (g + 1)],
                    in_=ob[:m])

    nc.compile()
    return nc


def kernel(**inputs) -> np.ndarray:
    global _last_results, _last_nc, _last_in_maps
    in_maps, meta = _host_prep(**inputs)
    Rp, Ro, Ap = meta["Rp"], meta["Ro"], meta["Ap"]

    nc = build_bass(Rp, Ro, Ap)
    _last_nc, _last_in_maps = nc, in_maps

    from concourse.bass_utils import run_bass_kernel_spmd
    res = run_bass_kernel_spmd(nc, in_maps, core_ids=list(range(NCORES)))
    _last_results = res

    out = np.zeros((NR, NA, H), np.float32)
    for c in range(B):
        rc, ac = int(meta["r_cnt"][c]), int(meta["a_cnt"][c])
        if rc == 0 or ac == 0:
            continue
        blk = res.results[c]["out"].reshape(Ro, Ap, H)
        r0, a0 = int(meta["r0"][c]), int(meta["a0"][c])
        out[r0:r0 + rc, a0:a0 + ac, :] = blk[:rc, :ac, :].astype(np.float32)
    return out
